# revision 6
# baseline (speedup 1.0000x reference)
"""Trainium2 Bass kernel: 7x7 valid cross-correlation (Conv2D) + bias on a
4096x4096 fp32 image, column-sharded over 8 NeuronCores (512 output columns
each, with a 6-column halo in each core's input slice).

Design (v3):
  - Column sharding: each core sees all 4096 input rows x 518 cols. Row
    tiles of 122 output rows (kin=128 input rows) give 34 tiles with only
    1.4% quantization waste (vs 19% for a row-sharded 512/122 split).
  - Input is cast to bf16 host-side: halves input HBM traffic and the PE
    streams bf16 at 1 col/cycle with no on-device cast.
  - The 2D conv is 7 accumulating matmuls per tile (one per horizontal tap
    b): psum[m, n] += B_b.T @ x[:, n+b], with B_b[k, m] = w[k-m, b] a
    banded [128 x 128] matrix doing the 7-tap vertical convolution.
    Bands are padded to 128 output rows so every matmul/evac/DMA touches
    all 128 partitions (uniform shapes; rows 122-127 are scratch).
  - DMA partition counts are all 128 for SDMA engine spray: the HWDGE
    splits a transfer's descriptors across (largest divisor of the
    partition count <= 16) engines at ~27 GB/s each; 122 = 2*61 would
    serialize on 2 engines.
  - Output DRAM is scratch-padded to [34*128, 512] so each tile is one
    full 128-partition write; the host gathers the valid 122 rows per
    tile. Outputs issue on the Activation HWDGE ring (nc.scalar), inputs
    on the SP ring (nc.sync), halving the serial DMA-issue load per ring.
  - Output stays fp32 (write bandwidth has headroom; halves rounding err).
"""

import sys

sys.path.insert(0, "/opt/trn_rl_repo")

import numpy as np

import concourse.bass as bass
import concourse.bacc as bacc
import concourse.mybir as mybir
from concourse.tile import TileContext
from concourse.bass_utils import run_bass_kernel_spmd

KH, KW = 7, 7
H, W = 4096, 4096
OH, OW = H - KH + 1, W - KW + 1  # 4090, 4090

NCORES = 8
CORE_OC = 512                     # output cols per core (core 7: 506 valid)
CORE_IC = CORE_OC + KW - 1        # 518 input cols per core
TILE_R = 122                      # valid output rows per tile
MROWS = 128                       # psum/output rows per tile (122 + 6 scratch)
N_TILES = -(-OH // TILE_R)        # 34 (33x122 + 64)

_NC_CACHE = {}


def _build_nc():
    f32 = mybir.dt.float32
    bf16 = mybir.dt.bfloat16

    nc = bacc.Bacc()
    x_in = nc.declare_dram_parameter("x_in", [H, CORE_IC], bf16, isOutput=False)
    bands = nc.declare_dram_parameter("bands", [128, KW * MROWS], bf16, isOutput=False)
    biasb = nc.declare_dram_parameter("biasb", [128, 1], f32, isOutput=False)
    y_out = nc.declare_dram_parameter(
        "y_out", [N_TILES * MROWS, CORE_OC], f32, isOutput=True
    )

    with TileContext(nc) as tc:
        with (
            tc.tile_pool(name="const", bufs=1) as cpool,
            tc.tile_pool(name="io", bufs=6) as iopool,
            tc.tile_pool(name="yo", bufs=4) as ypool,
            tc.tile_pool(name="ps", bufs=6, space="PSUM") as ppool,
        ):
            band_sb = cpool.tile([128, KW * MROWS], bf16)
            bias_sb = cpool.tile([128, 1], f32)
            # band on the Activation HWDGE ring so it overlaps the first x
            # tile's load on the SP ring
            nc.scalar.dma_start(out=band_sb[:, :], in_=bands[:, :])
            nc.scalar.dma_start(out=bias_sb[:, :], in_=biasb[:, :])

            # Warm up the PE during the DMA preamble: HAM un-throttles
            # (1.2 -> 2.4 GHz) only after ~3.4us of sustained PE activity,
            # so burn idle preamble time on dummy matmuls (inputs are
            # whatever SBUF holds; the psum result is never read).
            warm_in = cpool.tile([128, CORE_OC], bf16)
            nc.vector.memset(warm_in[:, :], 0)
            warm_ps = ppool.tile([128, CORE_OC], f32, tag="ps")
            for _ in range(12):
                nc.tensor.matmul(
                    warm_ps[:, :],
                    lhsT=warm_in[:, :128],
                    rhs=warm_in[:, :CORE_OC],
                    start=True,
                    stop=True,
                )

            for t in range(N_TILES):
                r0 = t * TILE_R
                kh = min(128, H - r0)
                x_sb = iopool.tile([128, CORE_IC], bf16, tag="x")
                nc.sync.dma_start(out=x_sb[:kh, :], in_=x_in[r0 : r0 + kh, :])
                ps = ppool.tile([128, CORE_OC], f32, tag="ps")
                for b in range(KW):
                    nc.tensor.matmul(
                        ps[:, :],
                        lhsT=band_sb[:kh, b * MROWS : (b + 1) * MROWS],
                        rhs=x_sb[:kh, b : b + CORE_OC],
                        start=(b == 0),
                        stop=(b == KW - 1),
                    )
                y_sb = ypool.tile([128, CORE_OC], f32, tag="y")
                nc.vector.tensor_scalar_add(y_sb[:, :], ps[:, :], bias_sb[:, 0:1])
                nc.scalar.dma_start(
                    out=y_out[t * MROWS : (t + 1) * MROWS, :], in_=y_sb[:, :]
                )
    nc.compile()
    return nc


def _make_bands(weight):
    """B_b[k, m] = w[k-m, b] laid out as [128, KW*MROWS] (band b in cols
    [b*MROWS, (b+1)*MROWS)); columns m >= TILE_R stay zero (scratch rows)."""
    bands = np.zeros((128, KW * MROWS), np.float32)
    m = np.arange(TILE_R)
    for b in range(KW):
        for a in range(KH):
            bands[m + a, b * MROWS + m] = weight[a, b]
    return bands.astype(mybir.dt.np(mybir.dt.bfloat16))


def _shard_inputs(x, weight, bias):
    bands = _make_bands(weight)
    biasb = np.full((128, 1), np.float32(bias[0]), np.float32)
    xb = x.astype(mybir.dt.np(mybir.dt.bfloat16))
    in_maps = []
    for c in range(NCORES):
        c0 = c * CORE_OC
        cc = min(CORE_IC, W - c0)
        xt = np.zeros((H, CORE_IC), xb.dtype)
        xt[:, :cc] = xb[:, c0 : c0 + cc]
        in_maps.append({"x_in": xt, "bands": bands, "biasb": biasb})
    return in_maps


def _assemble(results):
    out = np.empty((OH, OW), np.float32)
    for c in range(NCORES):
        c0 = c * CORE_OC
        cc = min(CORE_OC, OW - c0)
        yc = results[c]["y_out"]
        for t in range(N_TILES):
            r0 = t * TILE_R
            h = min(TILE_R, OH - r0)
            out[r0 : r0 + h, c0 : c0 + cc] = yc[t * MROWS : t * MROWS + h, :cc]
    return out


def _get_nc():
    if "nc" not in _NC_CACHE:
        _NC_CACHE["nc"] = _build_nc()
    return _NC_CACHE["nc"]


def _run(x, weight, bias, **spmd_kwargs):
    x = np.ascontiguousarray(np.asarray(x), dtype=np.float32)
    weight = np.asarray(weight, dtype=np.float32)
    bias = np.asarray(bias, dtype=np.float32)
    in_maps = _shard_inputs(x, weight, bias)
    res = run_bass_kernel_spmd(_get_nc(), in_maps, list(range(NCORES)), **spmd_kwargs)
    return _assemble(res.results), res


def kernel(x, weight, bias):
    out, _ = _run(x, weight, bias)
    return out


# revision 7
# speedup vs baseline: 1.0118x; 1.0118x over previous
"""Trainium2 Bass kernel: 7x7 valid cross-correlation (Conv2D) + bias on a
4096x4096 fp32 image, column-sharded over 8 NeuronCores (512 output columns
each, with a 6-column halo in each core's input slice).

Design (v6):
  - Column sharding: each core sees all input rows x 518 cols. Row tiles
    of 122 output rows (kin=128 input rows) give 34 tiles with only 1.4%
    quantization waste. Input rows are zero-padded to 4160 host-side so
    every tile loads a uniform 128 input rows.
  - Input is cast to bf16 host-side: halves input HBM traffic and the PE
    streams bf16 at 1 col/cycle with no on-device cast.
  - The 2D conv is 7 accumulating matmuls per tile (one per horizontal tap
    b): psum[m, n] += B_b.T @ x[:, n+b], with B_b[k, m] = w[k-m, b] a
    banded [128 x 128] matrix doing the 7-tap vertical convolution.
    Bands are padded to 128 output rows so every matmul/evac/DMA touches
    all 128 partitions (rows 122-127 are scratch).
  - All DMAs use 128 SBUF partitions: the HWDGE sprays a transfer's
    descriptors across (largest divisor of the partition count <= 16)
    SDMA engines at ~27 GB/s each, so 128 -> all 16 engines.
  - Tiles are processed in pairs sharing one input DMA (a 3D access
    pattern loads both 128-row windows, 122 rows apart, into one SBUF
    tile) and one output DMA (two tiles' outputs side by side in SBUF,
    scattered to a [17,2,128,512] scratch-padded DRAM layout). This
    halves the serial DMA-issue load on the HWDGE rings and the
    semaphore traffic. Inputs issue on the SP ring (nc.sync), outputs on
    the Activation ring (nc.scalar).
  - A memset + 8 dummy matmuls at program start warm the PE's HAM clock
    gate (1.2 -> 2.4 GHz needs ~3.4us of sustained activity) during the
    framework's startup preamble, so real matmuls run at full rate
    immediately.
  - Output stays fp32 (write bandwidth has headroom; halves rounding err).
"""

import sys

sys.path.insert(0, "/opt/trn_rl_repo")

import numpy as np

import bass_rust
import concourse.bass as bass
import concourse.bacc as bacc
import concourse.mybir as mybir
from concourse.tile import TileContext
from concourse.bass_utils import run_bass_kernel_spmd

KH, KW = 7, 7
H, W = 4096, 4096
OH, OW = H - KH + 1, W - KW + 1  # 4090, 4090

NCORES = 8
CORE_OC = 512                     # output cols per core (core 7: 506 valid)
CORE_IC = CORE_OC + KW - 1        # 518 input cols per core
TILE_R = 122                      # valid output rows per tile
MROWS = 128                       # psum/output rows per tile (122 + 6 scratch)
N_TILES = -(-OH // TILE_R)        # 34 (33x122 + 64)
N_PAIRS = N_TILES // 2            # 17
H_PAD = 4160                      # >= 122*33 + 128, keeps tile loads uniform

_NC_CACHE = {}


def _build_nc():
    f32 = mybir.dt.float32
    bf16 = mybir.dt.bfloat16

    nc = bacc.Bacc()
    x_in = nc.declare_dram_parameter("x_in", [H_PAD, CORE_IC], bf16, isOutput=False)
    bands = nc.declare_dram_parameter("bands", [128, KW * MROWS], bf16, isOutput=False)
    biasb = nc.declare_dram_parameter("biasb", [128, 1], f32, isOutput=False)
    y_out = nc.declare_dram_parameter(
        "y_out", [N_PAIRS, 2, MROWS, CORE_OC], f32, isOutput=True
    )

    with TileContext(nc) as tc:
        with (
            tc.tile_pool(name="const", bufs=1) as cpool,
            tc.tile_pool(name="io", bufs=4) as iopool,
            tc.tile_pool(name="yo", bufs=3) as ypool,
            tc.tile_pool(name="ps", bufs=6, space="PSUM") as ppool,
        ):
            band_sb = cpool.tile([128, KW * MROWS], bf16)
            bias_sb = cpool.tile([128, 1], f32)
            # consts on the Activation HWDGE ring so they overlap the first
            # x load on the SP ring
            nc.scalar.dma_start(out=band_sb[:, :], in_=bands[:, :])
            nc.scalar.dma_start(out=bias_sb[:, :], in_=biasb[:, :])

            # Warm up the PE during the startup preamble: HAM un-throttles
            # (1.2 -> 2.4 GHz) only after ~3.4us of sustained PE activity.
            warm_in = cpool.tile([128, CORE_OC], bf16)
            nc.gpsimd.memset(warm_in[:, :], 0)
            warm_ps = ppool.tile([128, CORE_OC], f32, tag="ps")
            for _ in range(8):
                nc.tensor.matmul(
                    warm_ps[:, :],
                    lhsT=warm_in[:, :128],
                    rhs=warm_in[:, :CORE_OC],
                    start=True,
                    stop=True,
                )

            for q in range(N_PAIRS):
                r0 = 2 * q * TILE_R
                # one DMA loads both tiles' 128-row windows (122 rows apart)
                x2 = iopool.tile([128, 2 * CORE_IC], bf16, tag="x")
                src = x_in[r0 : r0 + MROWS, :].copy()
                src.ap = bass_rust.VecI64Pair(
                    [[CORE_IC, 128], [TILE_R * CORE_IC, 2], [1, CORE_IC]]
                )
                nc.sync.dma_start(out=x2[:, :], in_=src)

                y2 = ypool.tile([128, 2 * CORE_OC], f32, tag="y")
                for half in range(2):
                    ps = ppool.tile([128, CORE_OC], f32, tag="ps")
                    for b in range(KW):
                        nc.tensor.matmul(
                            ps[:, :],
                            lhsT=band_sb[:, b * MROWS : (b + 1) * MROWS],
                            rhs=x2[:, half * CORE_IC + b : half * CORE_IC + b + CORE_OC],
                            start=(b == 0),
                            stop=(b == KW - 1),
                        )
                    nc.vector.tensor_scalar_add(
                        y2[:, half * CORE_OC : (half + 1) * CORE_OC],
                        ps[:, :],
                        bias_sb[:, 0:1],
                    )
                nc.scalar.dma_start(
                    out=y_out[q].rearrange("b p c -> p b c"), in_=y2[:, :]
                )
    nc.compile()
    return nc


def _make_bands(weight):
    """B_b[k, m] = w[k-m, b] laid out as [128, KW*MROWS] (band b in cols
    [b*MROWS, (b+1)*MROWS)); columns m >= TILE_R stay zero (scratch rows)."""
    bands = np.zeros((128, KW * MROWS), np.float32)
    m = np.arange(TILE_R)
    for b in range(KW):
        for a in range(KH):
            bands[m + a, b * MROWS + m] = weight[a, b]
    return bands.astype(mybir.dt.np(mybir.dt.bfloat16))


def _shard_inputs(x, weight, bias):
    bands = _make_bands(weight)
    biasb = np.full((128, 1), np.float32(bias[0]), np.float32)
    xb = x.astype(mybir.dt.np(mybir.dt.bfloat16))
    in_maps = []
    for c in range(NCORES):
        c0 = c * CORE_OC
        cc = min(CORE_IC, W - c0)
        xt = np.zeros((H_PAD, CORE_IC), xb.dtype)
        xt[:H, :cc] = xb[:, c0 : c0 + cc]
        in_maps.append({"x_in": xt, "bands": bands, "biasb": biasb})
    return in_maps


def _assemble(results):
    out = np.empty((OH, OW), np.float32)
    for c in range(NCORES):
        c0 = c * CORE_OC
        cc = min(CORE_OC, OW - c0)
        yc = results[c]["y_out"]  # [N_PAIRS, 2, MROWS, CORE_OC]
        for t in range(N_TILES):
            r0 = t * TILE_R
            h = min(TILE_R, OH - r0)
            out[r0 : r0 + h, c0 : c0 + cc] = yc[t // 2, t % 2, :h, :cc]
    return out


def _get_nc():
    if "nc" not in _NC_CACHE:
        _NC_CACHE["nc"] = _build_nc()
    return _NC_CACHE["nc"]


def _run(x, weight, bias, **spmd_kwargs):
    x = np.ascontiguousarray(np.asarray(x), dtype=np.float32)
    weight = np.asarray(weight, dtype=np.float32)
    bias = np.asarray(bias, dtype=np.float32)
    in_maps = _shard_inputs(x, weight, bias)
    res = run_bass_kernel_spmd(_get_nc(), in_maps, list(range(NCORES)), **spmd_kwargs)
    return _assemble(res.results), res


def kernel(x, weight, bias):
    out, _ = _run(x, weight, bias)
    return out


# revision 9
# speedup vs baseline: 1.0357x; 1.0236x over previous
"""Trainium2 Bass kernel: 7x7 valid cross-correlation (Conv2D) + bias on a
4096x4096 fp32 image, column-sharded over 8 NeuronCores (512 output columns
each, with a 6-column halo in each core's input slice).

Design (v6):
  - Column sharding: each core sees all input rows x 518 cols. Row tiles
    of 122 output rows (kin=128 input rows) give 34 tiles with only 1.4%
    quantization waste. Input rows are zero-padded to 4160 host-side so
    every tile loads a uniform 128 input rows.
  - Input is cast to bf16 host-side: halves input HBM traffic and the PE
    streams bf16 at 1 col/cycle with no on-device cast.
  - The 2D conv is 7 accumulating matmuls per tile (one per horizontal tap
    b): psum[m, n] += B_b.T @ x[:, n+b], with B_b[k, m] = w[k-m, b] a
    banded [128 x 128] matrix doing the 7-tap vertical convolution.
    Bands are padded to 128 output rows so every matmul/evac/DMA touches
    all 128 partitions (rows 122-127 are scratch).
  - All DMAs use 128 SBUF partitions: the HWDGE sprays a transfer's
    descriptors across (largest divisor of the partition count <= 16)
    SDMA engines at ~27 GB/s each, so 128 -> all 16 engines.
  - Tiles are processed in pairs sharing one input DMA (a 3D access
    pattern loads both 128-row windows, 122 rows apart, into one SBUF
    tile) and one output DMA (two tiles' outputs side by side in SBUF,
    scattered to a [17,2,128,512] scratch-padded DRAM layout). This
    halves the serial DMA-issue load on the HWDGE rings and the
    semaphore traffic. Inputs issue on the SP ring (nc.sync), outputs on
    the Activation ring (nc.scalar).
  - A memset + 8 dummy matmuls at program start warm the PE's HAM clock
    gate (1.2 -> 2.4 GHz needs ~3.4us of sustained activity) during the
    framework's startup preamble, so real matmuls run at full rate
    immediately.
  - Output stays fp32 (write bandwidth has headroom; halves rounding err).
"""

import sys

sys.path.insert(0, "/opt/trn_rl_repo")

import numpy as np

import bass_rust
import concourse.bass as bass
import concourse.bacc as bacc
import concourse.mybir as mybir
from concourse.tile import TileContext
from concourse.bass_utils import run_bass_kernel_spmd

KH, KW = 7, 7
H, W = 4096, 4096
OH, OW = H - KH + 1, W - KW + 1  # 4090, 4090

NCORES = 8
CORE_OC = 512                     # output cols per core (core 7: 506 valid)
CORE_IC = CORE_OC + KW - 1        # 518 input cols per core
TILE_R = 122                      # valid output rows per tile
MROWS = 128                       # psum/output rows per tile (122 + 6 scratch)
N_TILES = -(-OH // TILE_R)        # 34 (33x122 + 64)
N_PAIRS = N_TILES // 2            # 17
H_PAD = 4160                      # >= 122*33 + 128, keeps tile loads uniform

_NC_CACHE = {}


def _build_nc():
    f32 = mybir.dt.float32
    bf16 = mybir.dt.bfloat16

    nc = bacc.Bacc()
    x_in = nc.declare_dram_parameter("x_in", [H_PAD, CORE_IC], bf16, isOutput=False)
    bands = nc.declare_dram_parameter("bands", [128, KW * MROWS], bf16, isOutput=False)
    biasb = nc.declare_dram_parameter("biasb", [128, 1], f32, isOutput=False)
    y_out = nc.declare_dram_parameter(
        "y_out", [N_PAIRS, 2, MROWS, CORE_OC], f32, isOutput=True
    )

    with TileContext(nc) as tc:
        with (
            tc.tile_pool(name="const", bufs=1) as cpool,
            tc.tile_pool(name="io", bufs=4) as iopool,
            tc.tile_pool(name="yo", bufs=3) as ypool,
            tc.tile_pool(name="ps", bufs=6, space="PSUM") as ppool,
        ):
            band_sb = cpool.tile([128, KW * MROWS], bf16)
            bias_sb = cpool.tile([128, 1], f32)
            # consts on the Activation HWDGE ring so they overlap the first
            # x load on the SP ring
            nc.scalar.dma_start(out=band_sb[:, :], in_=bands[:, :])
            nc.scalar.dma_start(out=bias_sb[:, :], in_=biasb[:, :])

            # Warm up the PE during the startup preamble: HAM un-throttles
            # (1.2 -> 2.4 GHz) only after ~3.4us of sustained PE activity.
            warm_in = cpool.tile([128, CORE_OC], bf16)
            nc.gpsimd.memset(warm_in[:, :], 0)
            warm_ps = ppool.tile([128, CORE_OC], f32, tag="ps")
            for _ in range(10):
                nc.tensor.matmul(
                    warm_ps[:, :],
                    lhsT=warm_in[:, :128],
                    rhs=warm_in[:, :CORE_OC],
                    start=True,
                    stop=True,
                )

            for q in range(N_PAIRS):
                r0 = 2 * q * TILE_R
                # one DMA loads both tiles' 128-row windows (122 rows apart)
                x2 = iopool.tile([128, 2 * CORE_IC], bf16, tag="x")
                src = x_in[r0 : r0 + MROWS, :].copy()
                src.ap = bass_rust.VecI64Pair(
                    [[CORE_IC, 128], [TILE_R * CORE_IC, 2], [1, CORE_IC]]
                )
                nc.sync.dma_start(out=x2[:, :], in_=src)

                y2 = ypool.tile([128, 2 * CORE_OC], f32, tag="y")
                for half in range(2):
                    ps = ppool.tile([128, CORE_OC], f32, tag="ps")
                    for b in range(KW):
                        nc.tensor.matmul(
                            ps[:, :],
                            lhsT=band_sb[:, b * MROWS : (b + 1) * MROWS],
                            rhs=x2[:, half * CORE_IC + b : half * CORE_IC + b + CORE_OC],
                            start=(b == 0),
                            stop=(b == KW - 1),
                        )
                    nc.vector.tensor_scalar_add(
                        y2[:, half * CORE_OC : (half + 1) * CORE_OC],
                        ps[:, :],
                        bias_sb[:, 0:1],
                    )
                if q < N_PAIRS - 1:
                    nc.scalar.dma_start(
                        out=y_out[q].rearrange("b p c -> p b c"), in_=y2[:, :]
                    )
                else:
                    # last pair: per-tile writes so the first half drains
                    # while the second half is still computing, shortening
                    # the end-of-kernel DMA tail
                    nc.scalar.dma_start(out=y_out[q, 0], in_=y2[:, :CORE_OC])
                    nc.scalar.dma_start(out=y_out[q, 1], in_=y2[:, CORE_OC:])
    nc.compile()
    return nc


def _make_bands(weight):
    """B_b[k, m] = w[k-m, b] laid out as [128, KW*MROWS] (band b in cols
    [b*MROWS, (b+1)*MROWS)); columns m >= TILE_R stay zero (scratch rows)."""
    bands = np.zeros((128, KW * MROWS), np.float32)
    m = np.arange(TILE_R)
    for b in range(KW):
        for a in range(KH):
            bands[m + a, b * MROWS + m] = weight[a, b]
    return bands.astype(mybir.dt.np(mybir.dt.bfloat16))


def _shard_inputs(x, weight, bias):
    bands = _make_bands(weight)
    biasb = np.full((128, 1), np.float32(bias[0]), np.float32)
    xb = x.astype(mybir.dt.np(mybir.dt.bfloat16))
    in_maps = []
    for c in range(NCORES):
        c0 = c * CORE_OC
        cc = min(CORE_IC, W - c0)
        xt = np.zeros((H_PAD, CORE_IC), xb.dtype)
        xt[:H, :cc] = xb[:, c0 : c0 + cc]
        in_maps.append({"x_in": xt, "bands": bands, "biasb": biasb})
    return in_maps


def _assemble(results):
    out = np.empty((OH, OW), np.float32)
    for c in range(NCORES):
        c0 = c * CORE_OC
        cc = min(CORE_OC, OW - c0)
        yc = results[c]["y_out"]  # [N_PAIRS, 2, MROWS, CORE_OC]
        for t in range(N_TILES):
            r0 = t * TILE_R
            h = min(TILE_R, OH - r0)
            out[r0 : r0 + h, c0 : c0 + cc] = yc[t // 2, t % 2, :h, :cc]
    return out


def _get_nc():
    if "nc" not in _NC_CACHE:
        _NC_CACHE["nc"] = _build_nc()
    return _NC_CACHE["nc"]


def _run(x, weight, bias, **spmd_kwargs):
    x = np.ascontiguousarray(np.asarray(x), dtype=np.float32)
    weight = np.asarray(weight, dtype=np.float32)
    bias = np.asarray(bias, dtype=np.float32)
    in_maps = _shard_inputs(x, weight, bias)
    res = run_bass_kernel_spmd(_get_nc(), in_maps, list(range(NCORES)), **spmd_kwargs)
    return _assemble(res.results), res


def kernel(x, weight, bias):
    out, _ = _run(x, weight, bias)
    return out
